# revision 6
# baseline (speedup 1.0000x reference)
"""Trainium2 Bass kernel for nn_Attention_89077621719647.

Computes: out[b, d] = sum_t softmax(weights)[t] * (t < lengths[b]) * input[b, t, d]

Sharding: data-parallel over batch — each of the 8 cores handles 2 batch
rows (16 MiB of input).  The [T] weight vector is replicated.  Inside each
core the reduction over t is done on the TensorEngine: for each 128-row
t-chunk, a matmul with the (masked, exp-weighted) coefficient column as
the stationary operand accumulates into PSUM.  The softmax normalizer is
applied once at the end (scores = exp(w)/Z with a global Z, so masking and
normalization commute).
"""

import numpy as np

import concourse.bass as bass
import concourse.tile as tile
from concourse import bacc, mybir
from concourse.bass_utils import run_bass_kernel_spmd

B, T, D = 16, 2048, 1024
NCORES = 8
RPC = B // NCORES          # batch rows per core
NCHUNK = T // 128          # 16 t-chunks per row
F32 = mybir.dt.float32
F32R = mybir.dt.float32r

# tunables
TPD = 256                  # t-rows per DMA (256 rows x 4KiB = 1 MiB)
USE_F32R = True            # full-rate fp32 matmul mode
XPOOL_BUFS = 4


def _build_program(use_f32r=USE_F32R, tpd=TPD):
    nc = bacc.Bacc(
        "TRN2", target_bir_lowering=False, debug=False, num_devices=NCORES
    )
    x = nc.dram_tensor("x", [RPC, T, D], F32, kind="ExternalInput").ap()
    w = nc.dram_tensor("w", [T], F32, kind="ExternalInput").ap()
    lens = nc.dram_tensor("lens", [RPC, 128, 1], F32, kind="ExternalInput").ap()
    out = nc.dram_tensor("out", [RPC, D], F32, kind="ExternalOutput").ap()

    jt = tpd // 128        # t-chunks per DMA
    ng = T // tpd          # DMA groups per row

    with tile.TileContext(nc) as tc:
        with (
            tc.tile_pool(name="consts", bufs=1) as consts,
            tc.tile_pool(name="xin", bufs=XPOOL_BUFS) as xpool,
            tc.tile_pool(name="outs", bufs=2) as opool,
            tc.tile_pool(name="psum", bufs=4, space="PSUM") as pacc,
            tc.tile_pool(name="psumz", bufs=1, space="PSUM") as pz,
        ):
            # --- coefficient prep (tiny) ---
            # Wt[p, c] = w[c*128 + p]
            wt = consts.tile([128, NCHUNK], F32)
            nc.sync.dma_start(out=wt, in_=w.rearrange("(c p) -> p c", p=128))

            # E = exp(Wt), esum[p] = sum_c E[p, c]
            e = consts.tile([128, NCHUNK], F32)
            esum = consts.tile([128, 1], F32)
            nc.scalar.activation(
                out=e, in_=wt, func=mybir.ActivationFunctionType.Exp,
                accum_out=esum,
            )

            # Z = sum_p esum[p]  (cross-partition sum via ones-matmul)
            ones = consts.tile([128, 1], F32)
            nc.vector.memset(ones, 1.0)
            psum_z = pz.tile([1, 1], F32)
            nc.tensor.matmul(psum_z, lhsT=ones, rhs=esum, start=True, stop=True)
            rz = consts.tile([1, 1], F32)
            nc.vector.reciprocal(rz, psum_z)

            # it[p, c] = c*128 + p  (the t index of each element)
            it = consts.tile([128, NCHUNK], F32)
            nc.gpsimd.iota(
                it, pattern=[[128, NCHUNK]], base=0, channel_multiplier=1,
                allow_small_or_imprecise_dtypes=True,
            )

            # lens_sb[p, r] = lengths[b0 + r]  (host-replicated across p)
            lens_sb = consts.tile([128, RPC], F32)
            for r in range(RPC):
                nc.sync.dma_start(out=lens_sb[:, r : r + 1], in_=lens[r])

            # C[p, r, c] = (t < len_r) * exp(w[t]),  t = c*128 + p
            coef = consts.tile([128, RPC, NCHUNK], F32R if use_f32r else F32)
            for r in range(RPC):
                nc.vector.scalar_tensor_tensor(
                    out=coef[:, r, :], in0=it, scalar=lens_sb[:, r : r + 1],
                    in1=e, op0=mybir.AluOpType.is_lt, op1=mybir.AluOpType.mult,
                )

            # --- main streaming loop ---
            xdt = F32R if use_f32r else F32
            for r in range(RPC):
                ps = [
                    pacc.tile([1, 512], F32, name=f"ps_r{r}_d{dh}", tag="ps")
                    for dh in range(2)
                ]
                for g in range(ng):
                    xt = xpool.tile([128, jt, D], xdt, name="xt", tag="xt")
                    src = x[r, g * tpd : (g + 1) * tpd, :].rearrange(
                        "(j p) d -> p j d", p=128
                    )
                    if use_f32r:
                        # SWDGE cast-DMA rounds fp32 -> fp32r in flight
                        nc.gpsimd.dma_start(out=xt, in_=src)
                    else:
                        nc.sync.dma_start(out=xt, in_=src)
                    for j in range(jt):
                        ct = g * jt + j
                        lhsT = coef[:, r, ct : ct + 1]
                        for dh in range(2):
                            rhs = xt[:, j, dh * 512 : (dh + 1) * 512]
                            nc.tensor.matmul(
                                ps[dh], lhsT=lhsT, rhs=rhs,
                                start=(ct == 0), stop=(ct == NCHUNK - 1),
                            )
                # scale by 1/Z and write out
                ot = opool.tile([1, D], F32)
                for dh in range(2):
                    nc.vector.tensor_scalar(
                        out=ot[:, dh * 512 : (dh + 1) * 512], in0=ps[dh],
                        scalar1=rz, scalar2=None, op0=mybir.AluOpType.mult,
                    )
                nc.sync.dma_start(out=out[r : r + 1, :], in_=ot)

    nc.compile()
    return nc


_cached_nc = None


def _get_program():
    global _cached_nc
    if _cached_nc is None:
        _cached_nc = _build_program()
    return _cached_nc


def kernel(input, lengths, weights):
    input = np.asarray(input, dtype=np.float32)
    lengths = np.asarray(lengths)
    weights = np.asarray(weights, dtype=np.float32)

    nc = _get_program()

    w_np = np.ascontiguousarray(weights)
    in_maps = []
    for c in range(NCORES):
        rows = input[c * RPC : (c + 1) * RPC]
        lens_rep = np.ascontiguousarray(
            np.broadcast_to(
                lengths[c * RPC : (c + 1) * RPC].astype(np.float32)[:, None, None],
                (RPC, 128, 1),
            )
        )
        in_maps.append(
            {
                "x": np.ascontiguousarray(rows),
                "w": w_np,
                "lens": lens_rep,
            }
        )

    res = run_bass_kernel_spmd(nc, in_maps, list(range(NCORES)))
    out = np.concatenate([res.results[c]["out"] for c in range(NCORES)], axis=0)
    return out.astype(np.float32)
